# revision 39
# baseline (speedup 1.0000x reference)
"""Trainium2 Bass kernel for the temporal-shift + motion-excitation module.

Computation (per batch item b):
  x_shift[t,c,h,w] = sum_k shift_w[c,0,k] * x[t+k-1,c,h,w]   (zero-padded in t)
  xm[t,h,w]        = mean_c x_shift[t,c,h,w]
  p1               = conv3d(xm, me_w, padding=1)              (3x3x3, single channel)
  out              = x_shift * (1 + sigmoid(p1))

Strategy: pure data parallel over B=8 across 8 NeuronCores. Per core the
full [5,64,112,112] fp32 input (16 MB) is staged in SBUF as one tensor
laid out [h(partitions=112), (t,c,w)]. The temporal shift (a per-channel
one-hot selection in practice) is applied inside the load DMAs as source
AP offsets. Channel means are per-t vector reduces. The 3x3x3 conv is
done as 9x2 accumulating matmuls with tri-diagonal [112,112] weight
matrices on the TensorEngine (the band structure provides h zero-padding
for free; t/w padding comes from a zero-bordered xm buffer). Sigmoid on
the scalar engine straight out of PSUM, then one broadcast multiply per t
and contiguous stores.
"""

import sys

import numpy as np

if "/opt/trn_rl_repo" not in sys.path:
    sys.path.insert(0, "/opt/trn_rl_repo")

T, C, H, W = 5, 64, 112, 112
B = 8
W2 = W + 2
T2 = T + 2
F32 = np.float32


def _selection_runs(shift_w: np.ndarray):
    """If every channel's 3-tap filter is a one-hot with weight 1.0, return
    maximal runs of consecutive channels sharing the same temporal offset,
    as [(c0, c1, delta)]. Otherwise None."""
    w = np.asarray(shift_w, dtype=np.float64).reshape(C, 3)
    deltas = []
    for c in range(C):
        nz = np.nonzero(w[c])[0]
        if len(nz) != 1 or w[c, nz[0]] != 1.0:
            return None
        deltas.append(int(nz[0]) - 1)
    runs = []
    c0 = 0
    for c in range(1, C + 1):
        if c == C or deltas[c] != deltas[c0]:
            runs.append((c0, c, deltas[c0]))
            c0 = c
    return runs


def _numpy_reference(faces, shift_w, me_w):
    """Straight NumPy implementation of the module (correctness fallback)."""
    b, t, c, h, w = faces.shape
    sw = np.asarray(shift_w, F32).reshape(c, 3)
    xp = np.pad(faces, ((0, 0), (1, 1), (0, 0), (0, 0), (0, 0)))
    x_shift = (
        sw[None, None, :, 0, None, None] * xp[:, 0:t]
        + sw[None, None, :, 1, None, None] * xp[:, 1 : t + 1]
        + sw[None, None, :, 2, None, None] * xp[:, 2 : t + 2]
    ).astype(F32)
    xm = x_shift.mean(axis=2)
    k = np.asarray(me_w, F32).reshape(3, 3, 3)
    xmp = np.pad(xm, ((0, 0), (1, 1), (1, 1), (1, 1)))
    p1 = np.zeros_like(xm)
    for dt in range(3):
        for dh in range(3):
            for dw in range(3):
                p1 += k[dt, dh, dw] * xmp[:, dt : dt + t, dh : dh + h, dw : dw + w]
    gate = 1.0 / (1.0 + np.exp(-p1))
    return (x_shift * (1.0 + gate[:, :, None])).astype(F32)


def _emit_kernel(tc, x_ap, y_ap, runs, taps, iters=1):
    """Emit the per-core program. x_ap/y_ap are [T,C,H,W] DRAM APs; runs is
    the temporal-shift selection structure; taps is the 3x3x3 conv kernel
    already divided by C (so channel sums, not means, feed it). iters > 1
    repeats the whole body (for steady-state timing); tile-slot reuse makes
    iterations serialize through their data dependencies."""
    import concourse.bass as bass
    from concourse import mybir

    nc = tc.nc
    f32 = mybir.dt.float32
    t2, w2 = T + 2, W + 2
    tsplit = (T + 1) // 2  # first PSUM piece covers t' in [0, tsplit)

    # Tri-diagonal weight matrices: A[(dt,dw)][h_in, h_out] = taps[dt, h_in-h_out+1, dw]
    mats = np.zeros((9, H, H), dtype=F32)
    for i, (dt, dw) in enumerate((a, b) for a in range(3) for b in range(3)):
        for dh in range(3):
            d = dh - 1  # h_in - h_out; np.diag k is (col - row) = -d
            mats[i] += taps[dt, dh, dw] * np.diag(np.ones(H - abs(d), dtype=F32), k=-d)
    cm_dram = nc.inline_tensor(mats, name="convmats")

    nruns = len(runs)

    with (
        tc.tile_pool(name="main", bufs=1) as pool,
        tc.tile_pool(name="psum", bufs=1, space="PSUM") as psum,
    ):
        xt = pool.tile([H, T * C * W], f32, tag="xt")  # raw input, [h, (t,c,w)]
        rs = pool.tile([H, nruns * t2 * W], f32, tag="rs")  # per-run channel sums
        xm = pool.tile([H, t2 * w2], f32, tag="xm")  # padded shifted sums, [h,(t,w)]
        gp = pool.tile([H, T * W], f32, tag="gp")  # 1 + sigmoid(p1), [h, (t,w)]
        cm = pool.tile([H, 9 * H], f32, tag="cm")  # conv matrices, [h_in, (i,h_out)]
        pA = psum.tile([H, tsplit * W], f32, tag="pA")
        pB = psum.tile([H, (T - tsplit) * W], f32, tag="pB")
        dps = psum.tile([1, 1], f32, tag="dps")  # dummy for PE wait absorption

        xtv = xt[:].rearrange("p (t c w) -> p t c w", t=T, c=C, w=W)
        rsv = rs[:].rearrange("p (g u w) -> p g u w", g=nruns, u=t2, w=W)
        xmv = xm[:].rearrange("p (t w) -> p t w", t=t2, w=w2)
        gpv = gp[:].rearrange("p (t w) -> p t w", t=T, w=W)
        cmv = cm[:].rearrange("p (i m) -> p i m", i=9, m=H)

        xs = x_ap.transpose([2, 0, 1, 3])  # [H, T, C, W] view of DRAM input
        ys = y_ap.transpose([2, 0, 1, 3])

        # --- constants + zero borders ---
        nc.sync.dma_start(out=cmv, in_=cm_dram.ap().transpose([1, 0, 2]))
        nc.vector.memset(xm[:], 0.0)
        nc.vector.memset(rs[:], 0.0)

        for _ in range(iters):
            _emit_iteration(
                nc, mybir, runs, xtv, rsv, xmv, gp, gpv, cmv, xs, ys,
                pA, pB, dps, tsplit
            )


def _emit_iteration(nc, mybir, runs, xtv, rsv, xmv, gp, gpv, cmv, xs, ys, pA, pB, dps, tsplit):
    nruns = len(runs)
    if True:  # preserve indentation of the moved block
        # --- one large load per t (unshifted), then per-run channel sums ---
        for t in range(T):
            nc.sync.dma_start(out=xtv[:, t, :, :], in_=xs[:, t, :, :])
        for t in range(T):
            for g, (c0, c1, d) in enumerate(runs):
                nc.vector.reduce_sum(
                    out=rsv[:, g, t + 1, :],
                    in_=xtv[:, t, c0:c1, :].transpose([0, 2, 1]),
                    axis=mybir.AxisListType.X,
                )

        # --- combine per-run sums with their temporal offsets into xm ---
        # xm[t'] = sum_g rs[g, t' + d_g]  (zero-padded via rs borders)
        xint = xmv[:, 1 : T + 1, 1 : W + 1]

        def rswin(g):
            d = runs[g][2]
            return rsv[:, g, 1 + d : T + 1 + d, :]

        if nruns == 1:
            nc.vector.tensor_copy(xint, rswin(0))
        else:
            nc.vector.scalar_tensor_tensor(
                xint, rswin(0), 1.0, rswin(1),
                op0=mybir.AluOpType.mult, op1=mybir.AluOpType.add,
            )
            for g in range(2, nruns):
                nc.vector.scalar_tensor_tensor(
                    xint, xint, 1.0, rswin(g),
                    op0=mybir.AluOpType.mult, op1=mybir.AluOpType.add,
                )

        # Sacrificial 1x1 matmul: absorbs the conv-matrix DMA wait on PE so
        # the first real matmul carries only the DVE wait (walrus codegen
        # here encodes at most one sync wait per instruction).
        nc.tensor.matmul(dps[:], cmv[:, 0, 0:1], cmv[:, 0, 0:1], start=True, stop=True)

        # --- 3x3x3 conv as 9 accumulating tri-diagonal matmuls per PSUM piece ---
        for out_ps, ta, tb in ((pA, 0, tsplit), (pB, tsplit, T)):
            for i, (dt, dw) in enumerate((a, b) for a in range(3) for b in range(3)):
                nc.tensor.matmul(
                    out_ps[:],
                    cmv[:, i, :],
                    xmv[:, ta + dt : tb + dt, dw : W + dw],
                    start=(i == 0),
                    stop=(i == 8),
                )

        # --- gate = 1 + sigmoid(p1) ---
        nc.scalar.activation(
            gpv[:, 0:tsplit, :],
            pA[:].rearrange("p (t w) -> p t w", t=tsplit, w=W),
            mybir.ActivationFunctionType.Sigmoid,
        )
        nc.scalar.activation(
            gpv[:, tsplit:T, :],
            pB[:].rearrange("p (t w) -> p t w", t=T - tsplit, w=W),
            mybir.ActivationFunctionType.Sigmoid,
        )
        nc.vector.tensor_scalar_add(gp[:], gp[:], 1.0)

        # --- apply gate; each shifted run writes its gated slice into the
        # output-t slot of xt, so every output plane becomes contiguous and
        # is stored with a single DMA. The u-iteration orders below make the
        # in-slot moves WAR-safe under DVE program order. Stores go on the
        # SWDGE ring so no DMA semaphore lane is ever reused (walrus here
        # encodes at most one sync wait per instruction). ---
        def emit_mul(u, tout, c0, c1):
            g = gpv[:, tout, :].unsqueeze(1).broadcast_to((H, c1 - c0, W))
            # scalar_tensor_tensor (op0=bypass) == elementwise mul, but on a
            # TensorScalarPtr struct, which tolerates 2 sync waits in codegen.
            nc.vector.scalar_tensor_tensor(
                xtv[:, tout, c0:c1, :], xtv[:, u, c0:c1, :], 0.0, g,
                op0=mybir.AluOpType.bypass, op1=mybir.AluOpType.mult,
            )

        for c0, c1, d in runs:
            if d == 0:
                for u in range(T):
                    emit_mul(u, u, c0, c1)
            elif d > 0:
                for u in range(d, T):  # ascending: write u-d after reading u-d
                    emit_mul(u, u - d, c0, c1)
            else:
                for u in range(T - 1 + d, -1, -1):  # descending
                    emit_mul(u, u - d, c0, c1)
        for c0, c1, d in runs:
            if d != 0:
                tb = T - 1 if d > 0 else 0
                # in-place multiply by 0 == memset, on a 2-wait-capable struct
                nc.vector.tensor_scalar_mul(
                    xtv[:, tb, c0:c1, :], xtv[:, tb, c0:c1, :], 0.0
                )
        for t in range(T):
            nc.gpsimd.dma_start(out=ys[:, t, :, :], in_=xtv[:, t, :, :])


def _strip_redundant_waits(nc, keep_types=()):
    """Remove semaphore waits that are provably redundant at runtime. The
    walrus codegen used here encodes at most one sync wait on most
    instruction structs, while Tile emits waits per its (intentionally
    conservative, simulator-matching) model. Two sound elisions on real HW:

    1. Same-engine program order: in-order engines (DVE/ACT/PE/SP) drain one
       op before issuing the next, so a wait on the engine's own semaphore
       already reached by earlier increments in its stream always passes.
    2. Transitive implication: if instruction X waits S1 >= v1, and the
       in-order-engine instruction whose increment brought S1 to v1 had
       itself (directly or transitively) observed S2 >= v2, then X's wait
       on S2 >= v2 is implied and can be dropped.

    POOL is excluded everywhere: its 8 Q7 cores complete out of order, so
    neither its program order nor its semaphore values identify which of
    its instructions finished. Validate the UNSTRIPPED program in CoreSim
    (whose race detector assumes the conservative model) before calling."""
    import re

    inorder = {"DVE", "Activation", "PE", "SP"}
    # Only Tile's monotone (inc-only) scheduling sems participate in the
    # knowledge model. Barrier gather/release sems are decremented and
    # reused — eliding their waits corrupts the protocol.
    tracked = re.compile(r"^(DVE|Activation|PE|Pool|SP|DMAHW\d+|DMASW\d+)_\d+$")

    def sem_owner(name):
        return (name or "").rsplit("_", 1)[0]

    from concourse import mybir

    ev_count = 0
    for f in nc.m.functions:
        for bb in f.blocks:
            know: dict = {}  # engine -> {sem_name: known min value}
            pubs: dict = {}  # sem_name -> list of (cum_value, snapshot dict)
            cum: dict = {}  # sem_name -> cumulative increments so far
            out_insts = []
            for ins in bb.instructions:
                eng = str(ins.engine).rsplit(".", 1)[-1]
                K = know.setdefault(eng, {})
                si = ins.sync_info
                if si is None:
                    out_insts.append(ins)
                    continue
                if si.on_wait and type(ins).__name__ not in keep_types:

                    def usable(w):
                        return (
                            w.wait_mode == "sem-ge-imm"
                            and w.ant_name
                            and w.wait_value is not None
                            and tracked.match(w.ant_name)
                        )

                    def fold(w):  # absorb a satisfied wait's implications into K
                        if K.get(w.ant_name, 0) < w.wait_value:
                            K[w.ant_name] = w.wait_value
                        for cv, snap in pubs.get(w.ant_name, ()):
                            if cv <= w.wait_value:
                                for s, v in snap.items():
                                    if K.get(s, 0) < v:
                                        K[s] = v

                    # keep publication-rich (in-order engine) sems first so
                    # their implications can elide the other waits
                    order = sorted(
                        si.on_wait,
                        key=lambda w: 0 if usable(w) and w.ant_name in pubs else 1,
                    )
                    kept = []
                    for w in order:
                        if usable(w) and K.get(w.ant_name, 0) >= w.wait_value:
                            continue  # implied: by program order or a kept wait
                        kept.append(w)
                        if usable(w):
                            fold(w)
                    # Catch-all: at most ONE wait may remain on the
                    # instruction itself; peel the rest off into standalone
                    # EventSemaphore waits on the same engine, just before.
                    while len(kept) > 1:
                        w = kept.pop(0)
                        ev_count += 1
                        out_insts.append(
                            mybir.InstEventSemaphore(
                                name=f"evw-strip-{ev_count}",
                                engine=ins.engine,
                                sync_info=mybir.SyncInfo(on_wait=[w], on_update=[]),
                            )
                        )
                    if len(kept) != len(si.on_wait):
                        si.on_wait = kept
                elif si.on_wait:
                    for w in si.on_wait:
                        if (
                            w.wait_mode == "sem-ge-imm"
                            and w.ant_name
                            and w.wait_value is not None
                            and tracked.match(w.ant_name)
                        ):
                            if K.get(w.ant_name, 0) < w.wait_value:
                                K[w.ant_name] = w.wait_value
                for u in si.on_update:
                    if not u.ant_name:
                        continue
                    if u.update_mode != "sem-inc" or not u.update_value:
                        # reset/decrement: drop all knowledge of this sem
                        for k in know.values():
                            k.pop(u.ant_name, None)
                        pubs.pop(u.ant_name, None)
                        cum.pop(u.ant_name, None)
                        continue
                    if not tracked.match(u.ant_name):
                        continue
                    cum[u.ant_name] = cum.get(u.ant_name, 0) + u.update_value
                    owner = sem_owner(u.ant_name)
                    if owner == eng and owner in inorder:
                        K[u.ant_name] = cum[u.ant_name]
                        pubs.setdefault(u.ant_name, []).append(
                            (cum[u.ant_name], dict(K))
                        )
                out_insts.append(ins)
            bb.instructions = out_insts


def _build_nc(runs, taps, strip=True, iters=1):
    import concourse.bass as bass
    import concourse.tile as tile
    from concourse import mybir

    nc = bass.Bass(
        "TRN2",
        target_bir_lowering=False,
        debug=False,
        enable_asserts=False,
        num_devices=B,
    )
    x = nc.dram_tensor("x", [T, C, H, W], mybir.dt.float32, kind="ExternalInput").ap()
    y = nc.dram_tensor("y", [T, C, H, W], mybir.dt.float32, kind="ExternalOutput").ap()
    with tile.TileContext(nc) as tc:
        _emit_kernel(tc, x, y, runs, taps, iters=iters)
    if strip:
        _strip_redundant_waits(nc)
    return nc


_NC_CACHE: dict = {}


def _get_nc(runs, taps):
    key = (tuple(runs), taps.tobytes())
    if key not in _NC_CACHE:
        _NC_CACHE[key] = _build_nc(runs, taps)
    return _NC_CACHE[key]


def _run(faces, shift_w, me_w, **spmd_kwargs):
    faces = np.ascontiguousarray(np.asarray(faces, dtype=F32))
    shift_w = np.asarray(shift_w, dtype=F32)
    me_w = np.asarray(me_w, dtype=F32)

    runs = None
    if faces.shape == (B, T, C, H, W) and shift_w.shape == (C, 1, 3) and me_w.size == 27:
        runs = _selection_runs(shift_w)
    if runs is None:
        return _numpy_reference(faces, shift_w, me_w), None

    from concourse.bass_utils import run_bass_kernel_spmd

    taps = (me_w.reshape(3, 3, 3) / C).astype(F32)
    nc = _get_nc(runs, taps)
    in_maps = [{"x": faces[b]} for b in range(B)]
    res = run_bass_kernel_spmd(nc, in_maps, list(range(B)), **spmd_kwargs)
    return np.stack([res.results[b]["y"] for b in range(B)], axis=0), res


def kernel(faces, shift_w, me_w):
    return _run(faces, shift_w, me_w)[0]
